# revision 94
# baseline (speedup 1.0000x reference)
"""Trainium2 Bass kernel for AttentionWithRoPE (B=2, S=2048, HID=2048, H=16, D=128).

Sharding (8 cores): tensor-parallel over heads x data-parallel over batch.
Core c handles batch c//4 and heads 4*(c%4) .. 4*(c%4)+4:
  - One fused phase A streams hT once: per 512-wide s-tile it runs the Q
    and K projection passes (stationary w, moving h; RoPE fused on the DVE
    reading projection PSUM with partition-shifted reads) AND the V pass
    (stationary h, moving wv) so hidden is loaded exactly once. All matmul
    operands are bf16 (full PE rate = 1 cycle/moving-row, like f32r at
    free>=256, but half the HBM traffic and SBUF footprint). Input DMAs are
    interleaved (wq chunk, h chunk, ...) so the first matmul starts ~3us in.
  - Causal attention per head in scores^T orientation ([k, q]): exp on
    ScalarE straight from PSUM with the 1/sqrt(D) scale folded in; diagonal
    blocks multiply a 0/1 upper-tri mask. The softmax denominator costs
    almost no PE: per 128-col q-chunk it is an ap=1 matmul (stationary =
    exp chunk, moving = ones column -> [q,1] PSUM accumulated over k), then
    PE-transpose + DVE reciprocal + one K=1 broadcast matmul replace the
    old per-chunk ones-vector colsum accumulation (~29us -> ~5us PE).
    Phase B is software-pipelined per 128-wide k-chunk (scores -> exp ->
    PV two chunks later) with the normalization of iteration i woven into
    iteration i+1's instruction stream so the PE never waits on the
    Act/DVE chain.
  - Output projection partial with the core's w_o column slice; the host
    sums the four partials per batch (the TP reduce).
"""
import numpy as np
from contextlib import ExitStack

import ml_dtypes
import concourse.bass as bass
import concourse.tile as tile
from concourse import bacc, mybir
from concourse.bass_utils import run_bass_kernel_spmd

B, S, HID = 2, 2048, 2048
H, D = 16, 128
NCORES = 8
NH = 4                 # heads per core
HC = HID // 128        # hid chunks
AST = 512              # phase-A max s-tile width
TILE_W = [512, 512, 512, 512]
TILE_O = [0, 512, 1024, 1536]
QT = 512               # phase-B q-tile width
NQT = S // QT
DSCALE = float(D) ** -0.5
F32 = mybir.dt.float32
BF16 = mybir.dt.bfloat16
BF16NP = ml_dtypes.bfloat16

_CACHED = {}


def _build_nc():
    nc = bacc.Bacc("TRN2", target_bir_lowering=False, debug=False,
                   num_devices=NCORES)
    hT = nc.dram_tensor("hT", [HID, S], BF16, kind="ExternalInput")
    wqT = nc.dram_tensor("wqT", [HID, NH * D], BF16, kind="ExternalInput")
    wkT = nc.dram_tensor("wkT", [HID, NH * D], BF16, kind="ExternalInput")
    wvT = nc.dram_tensor("wvT", [HID, NH * D], BF16, kind="ExternalInput")
    woT = nc.dram_tensor("woT", [NH * D, HID], BF16, kind="ExternalInput")
    cosT = nc.dram_tensor("cosT", [D, S], F32, kind="ExternalInput")
    sinS = nc.dram_tensor("sinS", [D, S], F32, kind="ExternalInput")
    tri = nc.dram_tensor("tri", [128, 128], BF16, kind="ExternalInput")
    ident = nc.dram_tensor("ident", [128, 128], BF16, kind="ExternalInput")
    onesc = nc.dram_tensor("onesc", [128, 1], BF16, kind="ExternalInput")
    sel = nc.dram_tensor("sel", [NH, QT], BF16, kind="ExternalInput")
    out = nc.dram_tensor("out", [S, HID], F32, kind="ExternalOutput")

    hT_r = hT.ap().rearrange("(hc p) s -> p hc s", p=128)
    wqT_r = wqT.ap().rearrange("(hc p) m -> p hc m", p=128)
    wkT_r = wkT.ap().rearrange("(hc p) m -> p hc m", p=128)
    wvT_r = wvT.ap().rearrange("(hc p) m -> p hc m", p=128)
    woT_r = woT.ap().rearrange("(g p) n -> p g n", p=128)

    with tile.TileContext(nc) as tc, ExitStack() as ctx:
        # ---- small constants (loaded after the phase-A weights; first
        # needed by phase B) ----
        constp = ctx.enter_context(tc.tile_pool(name="const", bufs=1))
        tri_sb = constp.tile([128, 128], BF16, tag="tri", name="tri")
        ident_sb = constp.tile([128, 128], BF16, tag="ident", name="ident")
        onesc_sb = constp.tile([128, 1], BF16, tag="onesc", name="onesc")
        sel_sb = constp.tile([NH, QT], BF16, tag="sel", name="sel")

        # Q^T/K^T stay resident in SBUF through attention
        qkp = ctx.enter_context(tc.tile_pool(name="qk", bufs=1))
        qsb = qkp.tile([128, NH, S], BF16, tag="qsb", name="qsb")
        ksb = qkp.tile([128, NH, S], BF16, tag="ksb", name="ksb")
        # V in natural [k, d] orientation, resident through phase B
        vp = ctx.enter_context(tc.tile_pool(name="v", bufs=1))
        v_sb = vp.tile([128, S // 128, NH * D], BF16, tag="vsb", name="vsb")

        # Rotating 3-slot pool (16KB/partition each) hosting, over time:
        #   slot0: wq -> at   slot1: wk -> wo   slot2: wv
        wpool = ctx.enter_context(tc.tile_pool(name="aw", bufs=3))
        wq_sb = wpool.tile([128, HC, NH * D], BF16, tag="w", name="wq")
        wk_sb = wpool.tile([128, HC, NH * D], BF16, tag="w", name="wk")
        wv_sb = wpool.tile([128, HC, NH * D], BF16, tag="w", name="wv")

        hpool = ctx.enter_context(tc.tile_pool(name="ah", bufs=2))
        hb_tiles = [hpool.tile([128, HC, TILE_W[st]], BF16, tag="h",
                               name=f"hb{st}")
                    for st in range(len(TILE_W))]
        cs_sb = qkp.tile([128, S], F32, tag="cs", name="cs")
        ss_sb = qkp.tile([128, S], F32, tag="ss", name="ss")

        # Tile-0 DMA stream, ordered to match the hc-major consumption of
        # the first Q pass: (wq block, h block) pairs at 4-hid-chunk
        # granularity — per block the PE consumes 3.4us of matmuls vs
        # 2.9us of transfer, so after the first block the stream is
        # PE-bound (finer granularity hits the 632ns/DMA HWDGE floor).
        sl0 = slice(0, TILE_W[0])
        # leading slices shrink so the first matmul starts ~2us earlier
        for h2 in (slice(0, 1), slice(1, 2), slice(2, 4),
                   slice(4, 6), slice(6, 8), slice(8, 10), slice(10, 12),
                   slice(12, 14), slice(14, 16)):
            nc.sync.dma_start(wq_sb[:, h2, :], wqT_r[:, h2, :])
            nc.sync.dma_start(hb_tiles[0][:, h2, :], hT_r[:, h2, sl0])
        nc.sync.dma_start(cs_sb[:, sl0], cosT.ap()[:, sl0])
        nc.sync.dma_start(ss_sb[:, sl0], sinS.ap()[:, sl0])
        for c in range(HC // 2):
            h2 = slice(2 * c, 2 * c + 2)
            nc.sync.dma_start(wk_sb[:, h2, :], wkT_r[:, h2, :])
        for c in range(HC // 2):
            h2 = slice(2 * c, 2 * c + 2)
            nc.sync.dma_start(wv_sb[:, h2, :], wvT_r[:, h2, :])
        nc.sync.dma_start(tri_sb[:], tri.ap())
        nc.sync.dma_start(ident_sb[:], ident.ap())
        nc.sync.dma_start(onesc_sb[:], onesc.ap())
        nc.sync.dma_start(sel_sb[:], sel.ap())

        # ============ Phase A: fused Q/K/V projection passes ============
        # Each pass runs hc-major with 4 PSUM groups open so the PE can
        # consume weight/hidden chunks the moment their DMA lands (tile 0
        # streams behind the loads). RoPE of a pass drains on the DVE
        # while the next pass's matmuls run; the 8-bank rotation maps
        # passes to alternating 4-bank halves.
        with ExitStack() as astack:
            ropep = astack.enter_context(tc.tile_pool(name="arope", bufs=1))
            psA = astack.enter_context(
                tc.tile_pool(name="apsA", bufs=8, space="PSUM"))

            def rope(pst, dsb, h, sl, w):
                # RoPE: out = x*cos + shift(x)*sin_signed. The
                # partition-shifted reads go straight to PSUM (walrus
                # requires equal base partitions only when BOTH operands
                # are in SBUF).
                tsin = ropep.tile([128, AST], F32, tag="tsin", name="tsin")
                nc.vector.tensor_tensor(
                    tsin[0:64, 0:w], pst[64:128, 0:w], ss_sb[0:64, sl],
                    mybir.AluOpType.mult)
                nc.vector.tensor_tensor(
                    tsin[64:128, 0:w], pst[0:64, 0:w], ss_sb[64:128, sl],
                    mybir.AluOpType.mult)
                tcos = ropep.tile([128, AST], F32, tag="tcos", name="tcos")
                nc.vector.tensor_tensor(
                    tcos[:, 0:w], pst[0:128, 0:w], cs_sb[:, sl],
                    mybir.AluOpType.mult)
                nc.vector.tensor_tensor(
                    dsb[:, h, sl], tcos[:, 0:w], tsin[:, 0:w],
                    mybir.AluOpType.add)

            for st, (w, off) in enumerate(zip(TILE_W, TILE_O)):
                sl = slice(off, off + w)
                hb = hb_tiles[st]
                last = st == len(TILE_W) - 1
                if st > 0:
                    for c in range(4):
                        h4 = slice(4 * c, 4 * c + 4)
                        nc.sync.dma_start(hb[:, h4, :], hT_r[:, h4, sl])
                    nc.sync.dma_start(cs_sb[:, sl], cosT.ap()[:, sl])
                    nc.sync.dma_start(ss_sb[:, sl], sinS.ap()[:, sl])
                # Q then K pass: stationary w column block, moving h.
                # The last tile runs head-major so each group's RoPE
                # drains early, freeing its PSUM bank before phase B
                # claims it; other tiles run hc-major to track the DMA
                # stream.
                for wsb, dsb in ((wq_sb, qsb), (wk_sb, ksb)):
                    if last:
                        for h in range(NH):
                            ps = psA.tile([128, AST], F32, tag="psA",
                                          name="psA")
                            for hc in range(HC):
                                nc.tensor.matmul(
                                    ps[0:128, 0:w],
                                    wsb[:, hc, h * D:(h + 1) * D],
                                    hb[:, hc, :],
                                    start=(hc == 0), stop=(hc == HC - 1),
                                )
                            rope(ps, dsb, h, sl, w)
                    else:
                        pss = [psA.tile([128, AST], F32, tag="psA",
                                        name="psA") for _ in range(NH)]
                        for hc in range(HC):
                            for h in range(NH):
                                nc.tensor.matmul(
                                    pss[h][0:128, 0:w],
                                    wsb[:, hc, h * D:(h + 1) * D],
                                    hb[:, hc, :],
                                    start=(hc == 0), stop=(hc == HC - 1),
                                    skip_group_check=True,
                                )
                        for h in range(NH):
                            rope(pss[h], dsb, h, sl, w)
                # V pass: stationary h column block, moving wv -> [s, n].
                # The last tile keeps one V group to cover its K
                # pass's RoPE drain at the A->B boundary; the other three
                # are deferred into phase B's qt0 iterations (only read
                # by qt3, Act-independent PE filler for qt0's
                # exp-latency stalls).
                if last:
                    ps = psA.tile([128, NH * D], F32, tag="psA",
                                  name="psAv")
                    for hc in range(HC):
                        nc.tensor.matmul(
                            ps[:],
                            hb[:, hc, 0:128],
                            wv_sb[:, hc, :],
                            start=(hc == 0), stop=(hc == HC - 1),
                        )
                    nc.scalar.copy(v_sb[:, off // 128, :], ps[:])
                else:
                    pss = [psA.tile([128, NH * D], F32, tag="psA",
                                    name="psAv") for _ in range(w // 128)]
                    for hc in range(HC):
                        for sc in range(w // 128):
                            nc.tensor.matmul(
                                pss[sc][:],
                                hb[:, hc, sc * 128:(sc + 1) * 128],
                                wv_sb[:, hc, :],
                                start=(hc == 0), stop=(hc == HC - 1),
                                skip_group_check=True,
                            )
                    for sc in range(w // 128):
                        nc.scalar.copy(
                            v_sb[:, off // 128 + sc, :], pss[sc][:])

        # A^T reuses wq's slot; w_o reuses wk's slot (prefetch during B)
        at_all = wpool.tile([128, NH, S], BF16, tag="w", name="at")
        wo_sb = wpool.tile([128, NH, HID], BF16, tag="w", name="wo")
        for g in range(NH):
            nc.sync.dma_start(wo_sb[:, g, :], woT_r[:, g, :])

        # ============ Phase B + C (attention with the output projection
        # woven in: the Act engine bounds attention throughput — exp costs
        # rows*0.83ns + 185ns/op — so phase-C matmul groups, which have no
        # Act dependency, are interleaved to keep the PE busy) ============
        with ExitStack() as bctx:
            expp = bctx.enter_context(tc.tile_pool(name="bexp", bufs=16))
            smallp = bctx.enter_context(tc.tile_pool(name="bsmall", bufs=2))
            outp = bctx.enter_context(tc.tile_pool(name="cout", bufs=3))
            psS = bctx.enter_context(
                tc.tile_pool(name="bpss", bufs=3, space="PSUM"))
            psPV = bctx.enter_context(
                tc.tile_pool(name="bpspv", bufs=2, space="PSUM"))
            psRB = bctx.enter_context(
                tc.tile_pool(name="bpsrb", bufs=1, space="PSUM"))
            psDen = bctx.enter_context(
                tc.tile_pool(name="bpsden", bufs=1, space="PSUM"))
            psC = bctx.enter_context(
                tc.tile_pool(name="bpsc", bufs=1, space="PSUM"))

            # phase-C scheduler: (sc, nt) output-projection psum groups
            # become eligible once all 4 heads of sc's q-window are
            # normalized
            c_queue = []
            c_ot = {}

            def emit_c_group(on_act, pool=None):
                sc, nt = c_queue.pop(0)
                ssl = bass.ts(sc, 128)
                nsl = bass.ts(nt, QT)
                pool, ptag = pool or (psC, "o")
                ps = pool.tile([128, QT], F32, tag=ptag, name="o")
                for g in range(NH):
                    nc.tensor.matmul(
                        ps[:],
                        at_all[:, g, ssl],
                        wo_sb[:, g, nsl],
                        start=(g == 0), stop=(g == NH - 1),
                        skip_group_check=True,
                    )
                if sc not in c_ot:
                    c_ot[sc] = outp.tile([128, HID], F32, tag="ot",
                                         name="ot")
                ot = c_ot[sc]
                if on_act:
                    nc.scalar.copy(ot[:, nsl], ps[:])
                else:
                    nc.vector.tensor_copy(ot[:, nsl], ps[:])
                # every group stores its own column block immediately: a
                # row can straddle the woven stream and the tail, so a
                # single full-row store at the last block could skip the
                # earlier blocks (and the per-block store keeps the final
                # DMA tail short)
                nc.sync.dma_start(out.ap()[ssl, nsl], ot[:, nsl])
                if nt == HID // QT - 1:
                    del c_ot[sc]

            # deferred normalization of the previous (h, qt) iteration,
            # emitted in four stages woven into the current iteration's
            # chunk stream (one stage per chunk slot, alternating engines
            # so neither PE nor DVE ever waits on the cross-engine chain)
            pending = []
            state = {}

            def norm_stage(stage):
                if not pending:
                    return
                qsl, h, qtn, pvps, denps = pending[0]
                if stage == 0:
                    den_sb = smallp.tile([128, NH], BF16, tag="dsb",
                                         name="dsb")
                    with nc.allow_low_precision(
                            reason="softmax denom to bf16 for transpose"):
                        nc.vector.tensor_copy(den_sb[:], denps[:])
                    state["den_sb"] = den_sb
                elif stage == 1:
                    # full-shape tile on the "rbc" tag so the transpose
                    # output shares that bank in rotation (PSUM pool tiles
                    # are bank-granular)
                    dtps = psRB.tile([128, 2 * QT], BF16, tag="rbc",
                                     name="dt")
                    nc.tensor.matmul(dtps[0:NH, 0:128], state["den_sb"],
                                     ident_sb[:], is_transpose=True,
                                     start=True, stop=True,
                                     skip_group_check=True)
                    state["dtps"] = dtps
                elif stage == 2:
                    rec = smallp.tile([NH, 128], BF16, tag="rec",
                                      name="rec")
                    dtps = state["dtps"]
                    with nc.allow_low_precision(
                            reason="softmax denom reciprocal to bf16"):
                        nc.vector.reciprocal(rec[:], dtps[0:NH, 0:128])
                    state["rec"] = rec
                else:
                    pending.pop(0)
                    # broadcast 1/den along partitions: per q-chunk, a
                    # selector row picks that chunk's reciprocal row
                    # (row-sliced moving operands would start at an
                    # unaligned partition, which the ISA forbids)
                    rbc = psRB.tile([128, QT], F32, tag="rbc", name="rbc")
                    for qc in range(QT // 128):
                        nc.tensor.matmul(
                            rbc[:, qc * 128:(qc + 1) * 128],
                            sel_sb[:, qc * 128:(qc + 1) * 128],
                            state["rec"][:],
                            start=True, stop=True,
                            skip_group_check=True)
                    at_t = smallp.tile([128, QT], F32, tag="att", name="att")
                    nc.vector.tensor_copy(at_t[:], pvps[:])
                    nc.vector.tensor_tensor(
                        at_all[:, h, qsl], at_t[:], rbc[:],
                        mybir.AluOpType.mult)
                    # heads run in order per q-window, so the last head's
                    # norm completes the window: its output-projection
                    # rows become weavable
                    if h == NH - 1:
                        for sc in range(4 * qtn, 4 * qtn + 4):
                            for nt in range(HID // QT):
                                c_queue.append((sc, nt))

            for qt in range(NQT):
                qsl = bass.ts(qt, QT)
                for h in range(NH):
                    nallow = 4 * qt + 4
                    lag = 5 if qt == 3 else 4 if qt == 2 else 3 if qt == 1 else 2
                    pvps = psPV.tile([128, QT], F32, tag="pv", name="pv")
                    denps = psDen.tile([128, NH], F32, tag="den", name="den")
                    ebs = []

                    def pv_den(kc):
                        lo, eb = ebs[kc]
                        nc.tensor.matmul(
                            pvps[:, lo:QT],
                            v_sb[:, kc, h * D:(h + 1) * D],
                            eb[:, lo:QT],
                            start=(kc == 0), stop=(kc == nallow - 1),
                            skip_group_check=True,
                        )

                    def den_emit():
                        # softmax denominator: ap=1 matmuls, stationary =
                        # exp chunk, moving = ones column -> den[q, 1].
                        # qc-major: accumulation groups sharing a PSUM
                        # bank corrupt each other if they interleave in
                        # time, so each column's group runs to completion
                        # before the next opens.
                        for qc in range(QT // 128):
                            for kc in range(4 * qt + qc + 1):
                                eb = ebs[kc][1]
                                nc.tensor.matmul(
                                    denps[:, qc:qc + 1],
                                    eb[:, qc * 128:(qc + 1) * 128],
                                    onesc_sb[:],
                                    start=(kc == 0),
                                    stop=(kc == 4 * qt + qc),
                                    skip_group_check=True,
                                )

                    for kc in range(nallow):
                        j = kc - 4 * qt
                        lo = max(0, 128 * j)
                        sps = psS.tile([128, QT], F32, tag="s", name="s")
                        nc.tensor.matmul(
                            sps[:, lo:QT],
                            ksb[:, h, kc * 128:(kc + 1) * 128],
                            qsb[:, h, qt * QT + lo:(qt + 1) * QT],
                            start=True, stop=True,
                            skip_group_check=True,
                        )
                        eb = expp.tile([128, QT], BF16, tag="e", name="e")
                        ebs.append((lo, eb))
                        nc.scalar.activation(
                            eb[:, lo:QT], sps[:, lo:QT],
                            mybir.ActivationFunctionType.Exp, scale=DSCALE)
                        if j >= 0:
                            nc.vector.tensor_tensor(
                                eb[:, lo:lo + 128], eb[:, lo:lo + 128],
                                tri_sb[:], mybir.AluOpType.mult)
                        # Act surplus this chunk: exp is rows*0.83+185ns
                        # vs the PE's scores+pv at rows*0.83
                        state["deficit"] = state.get("deficit", 0.0) \
                            + 185.0
                        if kc < 4:
                            norm_stage(kc)
                        elif state["deficit"] > 350 and c_queue:
                            # weave output-projection groups into the
                            # Act-bound attention stream (PE filler)
                            emit_c_group(on_act=False)
                            state["deficit"] -= 853
                        # PV lag: qt0 is exp-latency-bound (full lag);
                        # wide iterations use lag 3 so diagonal chunks'
                        # extra exp->tri-mask DVE hop stays off the PE's
                        # critical path
                        if qt > 0 and kc >= lag:
                            pv_den(kc - lag)
                    if qt == 0 and h < 3:
                        # deferred last-tile V group: pure-PE filler
                        # placed exactly where qt0 waits on its exps
                        # (psC is idle until weaving starts)
                        hb3 = hb_tiles[-1]
                        vo = (S - TILE_W[-1]) // 128 + 1 + h
                        ps = psC.tile([128, NH * D], F32, tag="o",
                                      name="vdef")
                        for hc in range(HC):
                            nc.tensor.matmul(
                                ps[:],
                                hb3[:, hc,
                                    (h + 1) * 128:(h + 2) * 128],
                                wv_sb[:, hc, :],
                                start=(hc == 0), stop=(hc == HC - 1),
                                skip_group_check=True,
                            )
                        nc.scalar.copy(v_sb[:, vo, :], ps[:])
                    if qt == 0:
                        pv_den(0)
                        pv_den(1)
                    for j in range(min(lag, nallow), 2, -1):
                        pv_den(nallow - j)
                    pv_den(nallow - 2)
                    if state.get("deficit", 0) > 500 and c_queue:
                        emit_c_group(on_act=False)
                        state["deficit"] -= 853
                    pv_den(nallow - 1)
                    den_emit()
                    pending.append((qsl, h, qt, pvps, denps))
                    # the woven norm costs the PE ~320ns per iteration
                    # (transpose + broadcast matmul)
                    state["deficit"] = state.get("deficit", 0.0) - 320.0
            # final flush enqueues the last q-window's rows via stage 3
            for stage in range(4):
                norm_stage(stage)
            # drain the remaining output-projection groups (pure PE tail;
            # copies alternate DVE/Act now that exp pressure is gone).
            # The final row block stores per column block so the last DMA
            # is short.
            # attention PSUM banks are free now; run the tail through the
            # 3-deep scores pool so groups pipeline past their copies
            i = 0
            while c_queue:
                emit_c_group(on_act=(i % 2 == 1), pool=(psS, "s"))
                i += 1

    nc.compile()
    return nc


def _prep_in_maps(hidden_states, cos, sin, w_qkv, w_o):
    hs = np.asarray(hidden_states, dtype=np.float32)
    cos = np.asarray(cos, dtype=np.float32)
    sin = np.asarray(sin, dtype=np.float32)
    w_qkv = np.asarray(w_qkv, dtype=np.float32)
    w_o = np.asarray(w_o, dtype=np.float32)

    wT = np.ascontiguousarray(w_qkv.T).astype(BF16NP)   # (HID, 3*H*D)
    woTf = np.ascontiguousarray(w_o.T).astype(BF16NP)   # (H*D, HID)
    cosT = np.ascontiguousarray(cos.T)                  # (D, S)
    sinT = np.ascontiguousarray(sin.T)
    sinS = sinT.copy()
    sinS[:64] = -sinT[:64]
    tri = np.triu(np.ones((128, 128), BF16NP))
    ident = np.eye(128).astype(BF16NP)
    sel = np.zeros((NH, QT), BF16NP)
    for qc in range(QT // 128):
        sel[qc, qc * 128:(qc + 1) * 128] = 1

    hT = [np.ascontiguousarray(hs[b].T).astype(BF16NP) for b in range(B)]

    in_maps = []
    for c in range(NCORES):
        b, hg = c // 4, c % 4
        lo, hi = hg * NH * D, (hg + 1) * NH * D
        in_maps.append({
            "hT": hT[b],
            "wqT": np.ascontiguousarray(wT[:, lo:hi]),
            "wkT": np.ascontiguousarray(wT[:, H * D + lo:H * D + hi]),
            "wvT": np.ascontiguousarray(wT[:, 2 * H * D + lo:2 * H * D + hi]),
            "woT": np.ascontiguousarray(woTf[lo:hi, :]),
            "cosT": cosT,
            "sinS": sinS,
            "tri": tri,
            "ident": ident,
            "onesc": np.ones((128, 1), BF16NP),
            "sel": sel,
        })
    return in_maps


def kernel(hidden_states, cos, sin, w_qkv, w_o, _trace=False):
    if "nc" not in _CACHED:
        _CACHED["nc"] = _build_nc()
    nc = _CACHED["nc"]
    in_maps = _prep_in_maps(hidden_states, cos, sin, w_qkv, w_o)
    res = run_bass_kernel_spmd(nc, in_maps, core_ids=list(range(NCORES)),
                               trace=_trace)
    _CACHED["last_result"] = res
    out = np.zeros((B, S, HID), np.float32)
    for c in range(NCORES):
        out[c // 4] += res.results[c]["out"]
    return out
